# revision 11
# baseline (speedup 1.0000x reference)
"""Ewald real-space potential kernel for Trainium2 (8 NeuronCores, SPMD).

pot = C * sum_{i<j} q_i q_j erf(d_ij/sqrt(2)) / d_ij,  C = 90.0474/(2*pi).

V2 design (vs the 57us baseline): erf(d)/d is split as
    f(d2) = rinv(d2) + [g(d2) - rinv(d2)]
where rinv = 1/sqrt(d2) and g(s) = erf(sqrt(s/2))/sqrt(s).  The bracket is
non-negligible only for d < 3 (residual 2e-4 rel), so:

  * MAIN term (dense, all pairs once via cyclic half-window): per core a
    [4096-row x 512-col] slice becomes 20 j-blocks of [128, 512]:
      PE   : d2 via K=16 fp16 hi/lo split matmul (exact products)
      ACT  : rinv = Abs_reciprocal_sqrt(d2)  -- ONE pass, no Sqrt+recip chain
      DVE  : triangular boundary masks on 8 of 20 blocks
             (iota-vs-threshold compare fused with the mask multiply)
      PE   : s_i += q_j^T @ rinv via float32r matmuls (1 cyc/row, no cast)
    Pairs with cyclic offset exactly N/2 are excluded by the strict masks.
  * CORRECTION (sparse): host gathers close pairs (d<3, cell-binned) plus the
    N/2-offset pairs into [128, Fc] tiles; device computes d2 exactly from
    coordinate differences, g via a degree-8 polynomial (max err 3e-7), and
    accumulates qq*(sgA*g + sgR*rinv) with a fused tensor_tensor_reduce.
    The DIAG_EPS bias applied to same-window main-term pairs is mirrored
    here (sgE) so the subtraction cancels exactly.

Host sums the 8 per-core partial potentials.
"""

import numpy as np

try:
    from ml_dtypes import bfloat16 as _bf16
except ImportError:  # pragma: no cover
    _bf16 = np.float16

P = 128
N = 4096
NCORES = 8
COLS = N // NCORES          # 512 i-columns per core
HALF = N // 2               # cyclic half window (2048)
WINB = (HALF + COLS) // P   # 20 j-blocks per core
WIN = WINB * P              # 2560 j-rows per core
K = 16                      # split-matmul contraction rows
NORM_CONST = 90.0474 / (2.0 * np.pi)
DIAG_EPS = 0.01             # |r_j|^2 bias on same-window (u<512) j-slots
SUPERS = [3, 3, 3, 3, 3, 3, 2]   # j-blocks per ACT op / PSUM tile
S0 = 9.0                    # correction poly fitted on d2 in [0, S0]
RCUT2 = 9.0                 # close-pair cutoff d^2 (d < 3)

# degree-8 polynomial for g(s)=erf(sqrt(s/2))/sqrt(s), s = 9*t, t in [0,1]
# (computed once at import via chebfit; hardcoded to keep kernel.py hermetic)
GPOLY = [
    0.7978844518369046, -1.1968087108650778, 1.6152169061965957,
    -1.725778730983239, 1.4856357385139436, -1.0242578960663158,
    0.5329793165672329, -0.18231173586086336, 0.029874151355292762,
]
_CACHE = {}


def _fit_gpoly():
    from scipy.special import erf as _erf
    t = (np.cos(np.linspace(0, np.pi, 4001)) + 1.0) / 2.0
    s = np.maximum(t * S0, 1e-30)
    g = np.where(s > 1e-20, _erf(np.sqrt(s / 2.0)) / np.sqrt(s),
                 np.sqrt(2.0 / np.pi))
    c = np.polynomial.chebyshev.Chebyshev.fit(t, g, 8, domain=[0, 1])
    return c.convert(kind=np.polynomial.Polynomial).coef


def _split2(v32):
    h = v32.astype(np.float16)
    l = (v32 - h.astype(np.float32)).astype(np.float16)
    return h, l


def _close_pairs(r):
    """Unordered close pairs (d2 < RCUT2) via cell binning, canonicalized so
    (b - a) % N lies in [1, HALF]."""
    from scipy.spatial import cKDTree
    pairs = cKDTree(r).query_pairs(np.sqrt(RCUT2), output_type='ndarray')
    if len(pairs) == 0:
        return np.zeros((0,), np.int64), np.zeros((0,), np.int64)
    a, b = pairs[:, 0].astype(np.int64), pairs[:, 1].astype(np.int64)
    delta = (b - a) % N
    flip = delta > HALF
    aa = np.where(flip, b, a)
    bb = np.where(flip, a, b)
    return aa, bb


def _build_core_inputs(q, r, fc):
    q = q.astype(np.float32)
    r = r.astype(np.float32)
    r2_64 = (r.astype(np.float64) ** 2).sum(1)

    # ---- correction pair lists (shared across cores, dealt contiguously) --
    ca, cb = _close_pairs(r)                       # A-list: close pairs
    ba = np.arange(HALF, dtype=np.int64)           # B-list: offset-N/2 pairs
    bb = ba + HALF
    ia = np.concatenate([ca, ba])
    ib = np.concatenate([cb, bb])
    sgA = np.concatenate([np.ones(len(ca), np.float32),
                          np.zeros(len(ba), np.float32)])
    sgR = np.concatenate([-np.ones(len(ca), np.float32),
                          np.ones(len(ba), np.float32)])
    # eps flag: main term biased d2 by DIAG_EPS iff rotated row u = v+delta
    # < COLS (same-window leading blocks)
    delta = (ib - ia) % N
    v = ia % COLS
    sgE = ((v + delta) < COLS).astype(np.float32)
    npairs = len(ia)
    cap = NCORES * P * fc
    assert npairs <= cap, f"correction overflow: {npairs} > {cap}"
    # pad with a harmless real pair at zero weight (d2 > 0 keeps rinv finite)
    pad = cap - npairs
    pa = ia[0] if len(ca) else 0
    pb = ib[0] if len(ca) else 1
    ia = np.concatenate([ia, np.full(pad, pa)])
    ib = np.concatenate([ib, np.full(pad, pb)])
    for arr in (sgA, sgR, sgE):
        pass
    sgA = np.concatenate([sgA, np.zeros(pad, np.float32)])
    sgR = np.concatenate([sgR, np.zeros(pad, np.float32)])
    sgE = np.concatenate([sgE, np.zeros(pad, np.float32)])
    qI = q[ia].copy()
    qJ = q[ib].copy()
    qI[npairs:] = 0.0
    qJ[npairs:] = 0.0
    rI = r[ia]                                     # [cap, 3]
    rJ = r[ib]

    # iota / thresholds for the triangular boundary masks (core-independent)
    iota = np.broadcast_to(np.arange(COLS, dtype=np.float32),
                           (P, COLS)).copy()
    thr = np.empty((P, 8), np.float32)
    pidx = np.arange(P, dtype=np.float32)
    for b in range(4):                              # leading blocks 0..3
        thr[:, b] = 128 * b + pidx                  # include iff v < thr
    for kk in range(4):                             # trailing blocks 16..19
        thr[:, 4 + kk] = 128 * kk + pidx            # include iff v > thr

    in_maps = []
    percore = P * fc
    for c in range(NCORES):
        perm = (COLS * c + np.arange(WIN)) % N      # j-slot u -> atom index
        win = slice(COLS * c, COLS * (c + 1))       # this core's i-window

        rows_j, rows_i = [], []
        for d in range(3):
            cj = r[perm, d]
            ui = (-2.0 * r[win, d]).astype(np.float32)
            jh, jl = _split2(cj)
            ih, il = _split2(ui)
            rows_j += [jh, jh, jl, jl]
            rows_i += [ih, il, ih, il]
        r2j = r2_64[perm].copy()
        r2j[:COLS] += DIAG_EPS                      # same-window bias
        r2j = r2j.astype(np.float32)
        r2i = r2_64[win].astype(np.float32)
        jh, jl = _split2(r2j)
        ih, il = _split2(r2i)
        ones_j = np.ones(WIN, np.float16)
        ones_i = np.ones(COLS, np.float16)
        rows_j += [jh, jl, ones_j, ones_j]
        rows_i += [ones_i, ones_i, ih, il]

        qw = q[(COLS * c + np.arange(WIN)) % N].reshape(WINB, P).T.copy()

        sl = slice(c * percore, (c + 1) * percore)
        def tile(x):
            return np.ascontiguousarray(
                x[sl].reshape(fc, P).T.astype(np.float32))
        cp = np.concatenate([
            tile(rI[:, 0]), tile(rI[:, 1]), tile(rI[:, 2]),
            tile(rJ[:, 0]), tile(rJ[:, 1]), tile(rJ[:, 2]),
            tile(qI), tile(qJ), tile(sgA), tile(sgR), tile(sgE),
        ], axis=1)                                  # [P, 11*fc]

        in_maps.append({
            "aj": np.stack(rows_j).astype(np.float16),          # [K, WIN]
            "bi": np.stack(rows_i).astype(np.float16),          # [K, COLS]
            "qw": qw.astype(_bf16),                             # [P, WINB]
            "qi": (q[win] * NORM_CONST).reshape(1, COLS)
                  .astype(np.float32),                          # [1, COLS]
            "iota": iota,                                       # [P, COLS]
            "thr": thr,                                         # [P, 8]
            "cp": cp,                                           # [P, 11*fc]
        })
    return in_maps


def _build_program(fc):
    import concourse.mybir as mybir
    import concourse.tile as tile
    from concourse import bacc

    dt = mybir.dt
    alu = mybir.AluOpType
    rsq_fn = mybir.ActivationFunctionType.Abs_reciprocal_sqrt
    nc = bacc.Bacc("TRN2", target_bir_lowering=False, debug=False,
                   num_devices=NCORES)

    aj = nc.dram_tensor("aj", [K, WIN], dt.float16, kind="ExternalInput")
    bi = nc.dram_tensor("bi", [K, COLS], dt.float16, kind="ExternalInput")
    qw = nc.dram_tensor("qw", [P, WINB], dt.bfloat16, kind="ExternalInput")
    qi = nc.dram_tensor("qi", [1, COLS], dt.float32, kind="ExternalInput")
    iota = nc.dram_tensor("iota", [P, COLS], dt.float32, kind="ExternalInput")
    thr = nc.dram_tensor("thr", [P, 8], dt.float32, kind="ExternalInput")
    cp = nc.dram_tensor("cp", [P, 11 * fc], dt.float32, kind="ExternalInput")
    pot = nc.dram_tensor("pot", [1, 1], dt.float32, kind="ExternalOutput")

    a = [np.float32(x) for x in GPOLY]

    with tile.TileContext(nc) as tc:
        with (
            tc.tile_pool(name="const", bufs=1) as cpool,
            tc.tile_pool(name="rinvk", bufs=3) as rpool,
            tc.tile_pool(name="corr", bufs=1) as xpool,
            tc.tile_pool(name="d2pool", bufs=2, space="PSUM") as ppool,
            tc.tile_pool(name="spool", bufs=1, space="PSUM") as spool,
        ):
            BI = cpool.tile([K, COLS], dt.float16)
            nc.sync.dma_start(BI[:], bi[:])
            AJ = cpool.tile([K, WIN], dt.float16)
            nc.sync.dma_start(AJ[:, :WIN // 2], aj[:, :WIN // 2])
            nc.sync.dma_start(AJ[:, WIN // 2:], aj[:, WIN // 2:])
            QW = cpool.tile([P, WINB], dt.bfloat16)
            nc.sync.dma_start(QW[:], qw[:])
            QI = cpool.tile([1, COLS], dt.float32)
            nc.sync.dma_start(QI[:], qi[:])
            # --- PE warmup during the input DMA (p-state ramp) ---
            wsrc = cpool.tile([P, COLS], dt.float16)
            wl = cpool.tile([P, 1], dt.float16)
            nc.vector.memset(wsrc[:], 0.0)
            nc.vector.memset(wl[:], 0.0)
            IOTA = cpool.tile([P, COLS], dt.float32)
            nc.scalar.dma_start(IOTA[:], iota[:])
            THR = cpool.tile([P, 8], dt.float32)
            nc.scalar.dma_start(THR[:], thr[:])
            CP = cpool.tile([P, 11 * fc], dt.float32)
            nc.scalar.dma_start(CP[:], cp[:])

            s_psA = spool.tile([1, COLS], dt.float32)
            s_psB = spool.tile([1, COLS], dt.float32)

            for w in range(8):
                wp = s_psA if w % 2 == 0 else s_psB
                nc.tensor.matmul(wp[:, :], wl[:, :], wsrc[:, :],
                                 start=True, stop=True,
                                 skip_group_check=True)

            f32r = dt.float32r
            nsup = len(SUPERS)
            sup_base = np.cumsum([0] + SUPERS)

            d2_tiles = [None] * nsup
            rinv_tiles = [None] * nsup

            def emit_d2(g):
                gsz = SUPERS[g]
                d2 = ppool.tile([P, 3 * COLS], dt.float32, tag="d2")
                for kk in range(gsz):
                    jb = sup_base[g] + kk
                    nc.tensor.matmul(
                        d2[:, kk * COLS:(kk + 1) * COLS],
                        AJ[:, jb * P:(jb + 1) * P],
                        BI[:, :],
                        start=True, stop=True,
                    )
                d2_tiles[g] = d2

            def emit_rsqrt(g):
                gsz = SUPERS[g]
                rinv = rpool.tile([P, 3 * COLS], dt.bfloat16, tag="rinv")
                nc.scalar.activation(rinv[:, :gsz * COLS],
                                     d2_tiles[g][:, :gsz * COLS], rsq_fn)
                rinv_tiles[g] = rinv

            def emit_masks(g):
                gsz = SUPERS[g]
                rinv = rinv_tiles[g]
                for kk in range(gsz):
                    jb = sup_base[g] + kk
                    if jb < 4:          # leading: include iff v < 128*jb + p
                        op = alu.is_lt
                        tcol = jb
                    elif jb >= WINB - 4:  # trailing: include iff v > thr
                        op = alu.is_gt
                        tcol = 4 + (jb - (WINB - 4))
                    else:
                        continue
                    sl = slice(kk * COLS, (kk + 1) * COLS)
                    nc.vector.scalar_tensor_tensor(
                        out=rinv[:, sl], in0=IOTA[:, :],
                        scalar=THR[:, tcol:tcol + 1],
                        in1=rinv[:, sl],
                        op0=op, op1=alu.mult,
                    )

            def emit_reduce(g):
                gsz = SUPERS[g]
                rinv = rinv_tiles[g]
                for kk in range(gsz):
                    jb = sup_base[g] + kk
                    sl = slice(kk * COLS, (kk + 1) * COLS)
                    sp = s_psA if jb % 2 == 0 else s_psB
                    nc.tensor.matmul(
                        sp[:, :],
                        QW[:, jb:jb + 1],
                        rinv[:, sl],
                        start=(jb < 2), stop=(jb >= WINB - 2),
                        skip_group_check=True,
                    )

            # ---------- emission schedule ----------
            # PE:  warm, d2(0), d2(1), [red(g-2), d2(g)]..., red(5), red(6)
            # ACT: rsqrt(0..1), corr-rsqrt, rsqrt(2..6)
            # DVE: masks chase rsqrt; correction interleaved in the gaps.
            def corr_sl(m):
                return slice(m * fc, (m + 1) * fc)

            CT = xpool.tile([P, 10 * fc], dt.float32)   # correction scratch

            def emit_corr_d2():
                # dx,dy,dz -> scratch 0..2 ; d2p -> scratch 3
                for d in range(3):
                    nc.vector.tensor_tensor(
                        out=CT[:, corr_sl(d)], in0=CP[:, corr_sl(d)],
                        in1=CP[:, corr_sl(3 + d)], op=alu.subtract)
                for d in range(3):
                    nc.vector.tensor_tensor(
                        out=CT[:, corr_sl(d)], in0=CT[:, corr_sl(d)],
                        in1=CT[:, corr_sl(d)], op=alu.mult)
                nc.vector.tensor_tensor(
                    out=CT[:, corr_sl(3)], in0=CT[:, corr_sl(0)],
                    in1=CT[:, corr_sl(1)], op=alu.add)
                nc.vector.tensor_tensor(
                    out=CT[:, corr_sl(3)], in0=CT[:, corr_sl(3)],
                    in1=CT[:, corr_sl(2)], op=alu.add)
                # biased copy for rinv (matches main-term DIAG_EPS): -> 4
                nc.vector.scalar_tensor_tensor(
                    out=CT[:, corr_sl(4)], in0=CP[:, corr_sl(10)],
                    scalar=float(DIAG_EPS), in1=CT[:, corr_sl(3)],
                    op0=alu.mult, op1=alu.add)

            def emit_corr_rsqrt():
                # rinv_p -> 5
                nc.scalar.activation(CT[:, corr_sl(5)], CT[:, corr_sl(4)],
                                     rsq_fn)

            from concourse import bass_isa
            corr_acc = xpool.tile([P, 1], dt.float32)
            corr_tot = cpool.tile([P, 1], dt.float32)

            def emit_corr_poly():
                # t = d2p/S0 -> 6 ; Horner v -> 7
                nc.vector.tensor_scalar_mul(
                    CT[:, corr_sl(6)], CT[:, corr_sl(3)], float(1.0 / S0))
                nc.vector.tensor_scalar(
                    CT[:, corr_sl(7)], CT[:, corr_sl(6)],
                    float(a[8]), float(a[7]), alu.mult, alu.add)
                gammas = [0.0] + [float(a[m]) for m in range(6, 0, -1)]
                for gm in gammas:
                    nc.vector.scalar_tensor_tensor(
                        out=CT[:, corr_sl(7)], in0=CT[:, corr_sl(7)],
                        scalar=gm, in1=CT[:, corr_sl(6)],
                        op0=alu.add, op1=alu.mult)
                # u1 = (v + a0)*sgA -> 7 ; u2 = sgR*rinv_p -> 5
                nc.vector.scalar_tensor_tensor(
                    out=CT[:, corr_sl(7)], in0=CT[:, corr_sl(7)],
                    scalar=float(a[0]), in1=CP[:, corr_sl(8)],
                    op0=alu.add, op1=alu.mult)
                nc.vector.tensor_tensor(
                    out=CT[:, corr_sl(5)], in0=CT[:, corr_sl(5)],
                    in1=CP[:, corr_sl(9)], op=alu.mult)
                nc.vector.tensor_tensor(
                    out=CT[:, corr_sl(7)], in0=CT[:, corr_sl(7)],
                    in1=CT[:, corr_sl(5)], op=alu.add)
                # qq -> 8
                nc.vector.tensor_tensor(
                    out=CT[:, corr_sl(8)], in0=CP[:, corr_sl(6)],
                    in1=CP[:, corr_sl(7)], op=alu.mult)
                nc.vector.scalar_tensor_tensor(
                    out=CT[:, corr_sl(9)], in0=CT[:, corr_sl(7)],
                    scalar=float(NORM_CONST), in1=CT[:, corr_sl(8)],
                    op0=alu.mult, op1=alu.mult,
                    accum_out=corr_acc[:, :])

            emit_d2(0)
            emit_rsqrt(0)
            emit_masks(0)
            emit_d2(1)
            emit_rsqrt(1)
            emit_masks(1)
            emit_corr_d2()
            emit_corr_rsqrt()
            for g in range(2, nsup):
                emit_reduce(g - 2)
                emit_d2(g)
                emit_rsqrt(g)
                emit_masks(g)

            emit_reduce(nsup - 2)
            emit_reduce(nsup - 1)
            emit_corr_poly()
            nc.gpsimd.partition_all_reduce(
                corr_tot[:, :], corr_acc[:, :], channels=P,
                reduce_op=bass_isa.ReduceOp.add)

            # ---------- final reduction ----------
            sq = cpool.tile([1, COLS], dt.float32)
            sq2 = cpool.tile([1, COLS], dt.float32)
            pot_mainA = cpool.tile([1, 1], dt.float32)
            pot_mainB = cpool.tile([1, 1], dt.float32)
            nc.vector.scalar_tensor_tensor(
                out=sq[:, :], in0=s_psA[:, :], scalar=1.0, in1=QI[:, :],
                op0=mybir.AluOpType.mult, op1=mybir.AluOpType.mult,
                accum_out=pot_mainA[:, :],
            )
            nc.vector.scalar_tensor_tensor(
                out=sq2[:, :], in0=s_psB[:, :], scalar=1.0, in1=QI[:, :],
                op0=mybir.AluOpType.mult, op1=mybir.AluOpType.mult,
                accum_out=pot_mainB[:, :],
            )
            pot_main = cpool.tile([1, 1], dt.float32)
            nc.vector.tensor_tensor(pot_main[:, :], pot_mainA[:, :],
                                    pot_mainB[:, :], mybir.AluOpType.add)
            pot_sb = cpool.tile([1, 1], dt.float32)
            nc.vector.tensor_tensor(pot_sb[:, :], pot_main[:, :],
                                    corr_tot[0:1, :],
                                    mybir.AluOpType.add)
            nc.sync.dma_start(pot[:, :], pot_sb[:, :])

    nc.compile()
    return nc


def _get_program(fc):
    key = ("nc", fc)
    if key not in _CACHE:
        _CACHE[key] = _build_program(fc)
    return _CACHE[key]


def _run(q, r, trace=False, **trace_kwargs):
    from concourse.bass_utils import run_bass_kernel_spmd

    q = np.asarray(q)
    r = np.asarray(r)
    # capacity for close pairs + HALF offset pairs (recompiles only if the
    # default capacity of 32 free-slots per core is exceeded)
    ca, _ = _close_pairs(np.asarray(r, dtype=np.float32))
    need = len(ca) + HALF
    fc = 32
    while NCORES * P * fc < need:
        fc *= 2
    nc = _get_program(fc)
    in_maps = _build_core_inputs(q, r, fc)
    res = run_bass_kernel_spmd(nc, in_maps, core_ids=list(range(NCORES)),
                               trace=trace, **trace_kwargs)
    total = np.float64(0.0)
    for m in res.results:
        total += np.float64(m["pot"].reshape(-1)[0])
    return np.array([total], dtype=np.float32), res


def kernel(q, r, cell=None, batch=None):
    out, _ = _run(q, r, trace=False)
    return out
